# revision 16
# baseline (speedup 1.0000x reference)
"""AEV triplet-MLP kernel for Trainium2 (8 NeuronCores, batch-parallel).

Host (numpy): builds per-(center, pair) 9-feature MLP inputs over all 496
j<k pairs (invalid pairs masked via zero weight) and the smooth-cutoff
weights. Device (Bass/Tile): runs the 7-layer tanh MLP on every row and the
weighted pair-sum, one core per 4 batch items.

Row layout per core (4 batch items = 128 centers, 496 pairs each):
  centers grouped 8-at-a-time (16 groups); within a group rows are
  interleaved (p, j) -> row = p*8 + j so every 128-row chunk holds rows of
  all 8 centers (16 p-values x 8 lanes). 63488 rows = 496 chunks of 128 =
  31 iterations of 2048 rows (2 "blocks" of 1024).

MLP layout: activations [features, rows]; 64-wide layers packed two
512-row groups per 128 partitions with block-diagonal weights. Final
128->256 layer runs "flipped" (stationary = activation chunk) so its
output is [rows, features]; the weighted sum over pairs is then a K=128
matmul per chunk with an [128, 8] weight slice (one column per center
lane), accumulated in PSUM over a group's 31 chunks.
"""

import numpy as np

import concourse.bass as bass
import concourse.mybir as mybir
import concourse.tile as tile

CUTOFF = 3.5
EPS = 1e-7
B, N = 32, 32
NCORES = 8
BPC = B // NCORES            # batch items per core
CENTERS = BPC * N            # 128 centers per core
NPAIR = (N * (N - 1)) // 2   # 496 pairs (j < k), invalid ones zero-weighted
NGROUP = CENTERS // 8        # 16 groups of 8 centers
ROWS = CENTERS * NPAIR       # 63488 rows per core
NCHUNK = ROWS // 128         # 496
NITER = ROWS // 2048         # 31
NBLK = ROWS // 1024          # 62
CPG = NCHUNK // NGROUP       # 31 chunks per group
F32 = mybir.dt.float32
R32 = mybir.dt.float32r  # fp32 bytes, single-pass reduced-precision matmul

_PJ, _PK = np.triu_indices(N, k=1)  # pair index -> (j, k)

_CACHE = {}


def _host_features(D, Z):
    """D [B,N,N], Z [B,N] float32 -> x9 [B,N,NPAIR,9], w [B,N,NPAIR]."""
    jp, kp = _PJ, _PK
    R_ij = D[:, :, jp]                       # [B, N, P]
    R_ik = D[:, :, kp]
    R_jk = np.broadcast_to(D[:, jp, kp][:, None, :], R_ij.shape)
    z_i = np.broadcast_to(Z[:, :, None], R_ij.shape)
    z_j = np.broadcast_to(Z[:, jp][:, None, :], R_ij.shape)
    z_k = np.broadcast_to(Z[:, kp][:, None, :], R_ij.shape)

    def carnot(a, b, c):
        return (a * a + b * b - c * c) / np.maximum(2.0 * a * b, 1e-10)

    cos_i = carnot(R_ij, R_ik, R_jk)
    cos_j = carnot(R_ij, R_jk, R_ik)
    cos_k = carnot(R_ik, R_jk, R_ij)

    geo = np.stack(
        [R_ij + R_ik + R_jk,
         R_ij * R_ik + R_ij * R_jk + R_ik * R_jk,
         R_ij * R_ik * R_jk], axis=-1)
    chem = np.stack(
        [z_i + z_j + z_k,
         cos_i + cos_j + cos_k,
         z_i * (z_j + z_k) + z_j * z_k - cos_i * (cos_j + cos_k) - cos_j * cos_k,
         z_i * (cos_j + cos_k) + cos_i * (z_j + z_k) + z_j * cos_k + cos_j * z_k,
         z_i * (z_j * z_k - cos_j * cos_k) - cos_i * (z_j * cos_k + cos_j * z_k),
         z_i * (z_j * cos_k + cos_j * z_k) + cos_i * (z_j * z_k - cos_j * cos_k)],
        axis=-1)
    geo = geo / (np.linalg.norm(geo, axis=-1, keepdims=True).astype(np.float32) + EPS)
    chem = chem / (np.linalg.norm(chem, axis=-1, keepdims=True).astype(np.float32) + EPS)
    x9 = np.concatenate([geo, chem], axis=-1).astype(np.float32)

    def f_C(d):
        return 0.5 * np.cos(np.pi * d / CUTOFF) + 0.5

    within = (R_ij < CUTOFF) & (R_ik < CUTOFF)
    i_idx = np.arange(N)[None, :, None]
    valid = (jp[None, None, :] != i_idx) & (kp[None, None, :] != i_idx)
    w = (f_C(R_ij) * f_C(R_ik) * (within & valid)).astype(np.float32)
    return x9, w


def _core_rows(x9_core, w_core):
    """x9_core [CENTERS, P, 9], w_core [CENTERS, P] -> row-ordered arrays.

    Returns x9d [NBLK, 18, 512] (two 512-row groups stacked on partitions)
    and wredd [128, NCHUNK*8] (per-chunk [128, 8] reduction lhsT slices).
    """
    x9r = (x9_core.reshape(NGROUP, 8, NPAIR, 9)
           .transpose(0, 2, 1, 3).reshape(ROWS, 9))
    wr = (w_core.reshape(NGROUP, 8, NPAIR)
          .transpose(0, 2, 1).reshape(ROWS))

    t = x9r.reshape(NBLK, 2, 512, 9)
    x9d = np.ascontiguousarray(
        np.concatenate([t[:, 0], t[:, 1]], axis=2).transpose(0, 2, 1))

    wch = wr.reshape(NCHUNK, 128)
    M = np.zeros((NCHUNK, 128, 8), np.float32)
    q = np.arange(128)
    M[:, q, q % 8] = wch
    wredd = np.ascontiguousarray(M.transpose(1, 0, 2).reshape(128, NCHUNK * 8))
    return x9d.astype(np.float32), wredd


def _pack_weights(ws, bs):
    """ws/bs: lists of W0..W6, b0..b6 -> dict of device weight tensors."""
    W0, W1, W2, W3, W4, W5, W6 = ws
    b0, b1, b2, b3, b4, b5, b6 = bs

    w0d = np.zeros((18, 128), np.float32)
    w0d[0:9, 0:64] = W0
    w0d[9:18, 64:128] = W0

    wmid = np.zeros((128, 4, 128), np.float32)
    for li, Wl in enumerate([W1, W2, W3, W4]):
        wmid[0:64, li, 0:64] = Wl
        wmid[64:128, li, 64:128] = Wl

    w5g = np.zeros((128, 2, 128), np.float32)
    w5g[0:64, 0, :] = W5
    w5g[64:128, 1, :] = W5

    bvec = np.zeros((128, 6), np.float32)
    for li, bl in enumerate([b0, b1, b2, b3, b4]):
        bvec[0:64, li] = bl
        bvec[64:128, li] = bl
    bvec[:, 5] = b5

    b6b = np.ascontiguousarray(
        np.broadcast_to(np.concatenate([b6, b6]), (128, 512))).astype(np.float32)

    return {
        "w0d": w0d,
        "wmid": wmid,
        "w5g": w5g,
        "w6": np.ascontiguousarray(W6.astype(np.float32)),
        "bvec": bvec,
        "b6b": b6b,
    }


def _split_sync_waits(nc, max_waits):
    """Hoist excess per-instruction semaphore waits onto preceding NoOps.

    The installed walrus build rejects instructions carrying more than
    `max_waits` sync waits ("Too many sync wait commands"); waiting on the
    same engine queue via dedicated NoOps first is semantically identical.
    """
    n = 0
    for fn in nc.m.functions:
        for bb in fn.blocks:
            il = bb.instructions
            out = []
            for inst in il:
                si = inst.sync_info
                if si is not None and si.on_wait and len(si.on_wait) > max_waits:
                    waits = list(si.on_wait)
                    for w in waits[:-max_waits]:
                        nop = mybir.InstNoOp(
                            name=nc.get_next_instruction_name(),
                            sync_info=mybir.SyncInfo(on_wait=[w], on_update=[]),
                            bass_nofuse=True,
                            engine=inst.engine,
                        )
                        nc.register_instruction(nop)
                        out.append(nop)
                        n += 1
                    si.on_wait = waits[-max_waits:]
                out.append(inst)
            il[:] = out
    return n


def _build_nc(max_waits=1, mm_dtype=R32):
    nc = bass.Bass()
    Tanh = mybir.ActivationFunctionType.Tanh
    MD = mm_dtype

    def r(ap):
        return ap

    x9 = nc.dram_tensor("x9", [NBLK, 18, 512], MD, kind="ExternalInput")
    wred = nc.dram_tensor("wred", [128, NCHUNK * 8], MD, kind="ExternalInput")
    w0d = nc.dram_tensor("w0d", [18, 128], MD, kind="ExternalInput")
    wmid = nc.dram_tensor("wmid", [128, 4, 128], MD, kind="ExternalInput")
    w5g = nc.dram_tensor("w5g", [128, 2, 128], MD, kind="ExternalInput")
    w6 = nc.dram_tensor("w6", [128, 256], MD, kind="ExternalInput")
    bvec = nc.dram_tensor("bvec", [128, 6], F32, kind="ExternalInput")
    b6b = nc.dram_tensor("b6b", [128, 512], F32, kind="ExternalInput")
    out = nc.dram_tensor("out", [CENTERS, 256], F32, kind="ExternalOutput")

    with tile.TileContext(nc) as tc, \
         tc.tile_pool(name="const", bufs=1) as cpool, \
         tc.tile_pool(name="xin", bufs=4) as xpool, \
         tc.tile_pool(name="acts", bufs=3) as apool, \
         tc.tile_pool(name="blks", bufs=2) as bpool, \
         tc.tile_pool(name="blk3", bufs=2) as b3pool, \
         tc.tile_pool(name="t6", bufs=3) as t6pool, \
         tc.tile_pool(name="redsb", bufs=2) as rspool, \
         tc.tile_pool(name="mpsum", bufs=4, space="PSUM") as mpsum, \
         tc.tile_pool(name="lpsum", bufs=2, space="PSUM") as lpsum, \
         tc.tile_pool(name="rpsum", bufs=2, space="PSUM") as rpsum:

        w0_sb = cpool.tile([18, 128], MD)
        nc.sync.dma_start(w0_sb[:], w0d[:])
        wmid_sb = cpool.tile([128, 4, 128], MD)
        nc.sync.dma_start(wmid_sb[:], wmid[:])
        w5g_sb = cpool.tile([128, 2, 128], MD)
        nc.sync.dma_start(w5g_sb[:], w5g[:])
        w6_sb = cpool.tile([128, 256], MD)
        nc.sync.dma_start(w6_sb[:], w6[:])
        bv_sb = cpool.tile([128, 6], F32)
        nc.sync.dma_start(bv_sb[:], bvec[:])
        b6b_sb = cpool.tile([128, 512], F32)
        nc.sync.dma_start(b6b_sb[:], b6b[:])
        wred_sb = cpool.tile([128, NCHUNK * 8], MD)
        nc.sync.dma_start(wred_sb[:], wred[:])

        red_tiles = {}

        def emit_l6_mms(st, cc):
            # two flipped-L6 chunk matmuls; the b6 add is deferred a section
            blk3 = st["blk3"]
            p6 = lpsum.tile([128, 512], F32, tag="lp",
                            name=f"p6_{st['it']}_{cc}")
            c0, c1 = 2 * cc, 2 * cc + 1
            nc.tensor.matmul(p6[:, 0:256], blk3[:, c0 * 128:(c0 + 1) * 128],
                             w6_sb[:], start=True, stop=True)
            nc.tensor.matmul(p6[:, 256:512], blk3[:, c1 * 128:(c1 + 1) * 128],
                             w6_sb[:], start=True, stop=True)
            st["p6"][cc] = p6

        def emit_l6_add(st, cc):
            nc.vector.tensor_add(out=st["t6"][:, cc * 512:(cc + 1) * 512],
                                 in0=st["p6"].pop(cc)[:], in1=b6b_sb[:])

        def emit_red(st, c):
            g = st["it"] * 16 + c
            G, loc = g // CPG, g % CPG
            if loc == 0:
                red_tiles[G] = rpsum.tile([8, 256], F32, tag="red",
                                          name=f"red_{G}")
            rp = red_tiles[G]
            nc.tensor.matmul(rp[:], wred_sb[:, g * 8:(g + 1) * 8],
                             st["t6h"][:, c * 256:(c + 1) * 256],
                             start=(loc == 0), stop=(loc == CPG - 1),
                             skip_group_check=True)
            if loc == CPG - 1:
                rs = rspool.tile([8, 256], F32, tag="rs", name=f"rs_{G}")
                nc.vector.tensor_copy(rs[:], rp[:])
                nc.sync.dma_start(out[G * 8:(G + 1) * 8, :], rs[:])
                del red_tiles[G]

        # 4-stage software pipeline. Iter `it` emits:
        #   s0=it:   MLP matmuls/tanhs (residual adds on GpSimd)
        #   s1=it-1: flipped-L6 chunk matmuls + b6 adds (PE/DVE filler)
        #   s2=it-2: the big L6 tanh (leads the ACT queue, input fully ready)
        #   s3=it-3: weighted-reduction matmuls (PE filler)
        # so the in-order PE/ACT queues always hold independent work between
        # layer-dependent ops (keeps PE dense -> HAM warm).
        s1 = s2 = s3 = None
        for it in range(NITER + 3):
            mlp = it < NITER
            st = None
            xres = blk1 = blk2 = cur = None
            if mlp:
                st = {"it": it}
                kA, kB = 2 * it, 2 * it + 1
                xtA = xpool.tile([18, 512], MD, tag="xt", name=f"xtA_{it}")
                nc.sync.dma_start(xtA[:], x9[kA])
                xtB = xpool.tile([18, 512], MD, tag="xt", name=f"xtB_{it}")
                nc.sync.dma_start(xtB[:], x9[kB])

            if s2 is not None:
                t6h = t6pool.tile([128, 4096], MD, tag="t6h",
                                  name=f"t6h_{s2['it']}")
                nc.scalar.activation(t6h[:, 0:2048], s2["t6"][:, 0:2048], Tanh)
                s2["t6h"] = t6h

            if mlp:
                # L0: 9 -> 64 (two row-groups block-diagonal packed)
                p0a = mpsum.tile([128, 512], F32, tag="mp", name=f"p0a_{it}")
                nc.tensor.matmul(p0a[:], w0_sb[:], xtA[:], start=True, stop=True)
                p0b = mpsum.tile([128, 512], F32, tag="mp", name=f"p0b_{it}")
                nc.tensor.matmul(p0b[:], w0_sb[:], xtB[:], start=True, stop=True)
                xres = apool.tile([128, 1024], MD, tag="xres",
                                  name=f"xres_{it}")
                nc.scalar.activation(xres[:, 0:512], p0a[:], Tanh,
                                     bias=bv_sb[:, 0:1])
                nc.scalar.activation(xres[:, 512:1024], p0b[:], Tanh,
                                     bias=bv_sb[:, 0:1])
                cur = xres

            if s1 is not None:
                s1["p6"] = {}
                emit_l6_mms(s1, 0)
                emit_l6_mms(s1, 1)
            if s3 is not None:
                for c in range(0, 4):
                    emit_red(s3, c)

            # L1..L4: 64 -> 64, interleaved with filler stages
            first_li = True
            for li in range(4):
                if mlp:
                    pla = mpsum.tile([128, 512], F32, tag="mp",
                                     name=f"p{li + 1}a_{it}")
                    nc.tensor.matmul(pla[:], wmid_sb[:, li],
                                     cur[:, 0:512], start=True, stop=True)
                    plb = mpsum.tile([128, 512], F32, tag="mp",
                                     name=f"p{li + 1}b_{it}")
                    nc.tensor.matmul(plb[:], wmid_sb[:, li],
                                     cur[:, 512:1024], start=True, stop=True)
                    h = apool.tile([128, 1024], MD, tag="h", name=f"h{li}_{it}")
                    nc.scalar.activation(h[:, 0:512], pla[:], Tanh,
                                         bias=bv_sb[:, li + 1:li + 2])
                    nc.scalar.activation(h[:, 512:1024], plb[:], Tanh,
                                         bias=bv_sb[:, li + 1:li + 2])
                    if li == 0:
                        blk1 = bpool.tile([128, 1024], MD, tag="blk1",
                                          name=f"blk1_{it}")
                        nc.vector.tensor_add(out=blk1[:], in0=h[:], in1=xres[:])
                        cur = blk1
                    elif li < 3:
                        cur = h
                    else:
                        blk2 = bpool.tile([128, 1024], MD, tag="blk2",
                                          name=f"blk2_{it}")
                        nc.vector.tensor_add(out=blk2[:], in0=h[:], in1=blk1[:])
                        cur = blk2
                if first_li and s2 is not None:
                    # second half of the big L6 tanh fills ACT's L1 wait
                    nc.scalar.activation(s2["t6h"][:, 2048:4096],
                                         s2["t6"][:, 2048:4096], Tanh)
                first_li = False
                if s1 is not None:
                    if li < 2:
                        emit_l6_mms(s1, 2 * li + 2)
                        emit_l6_mms(s1, 2 * li + 3)
                    elif li == 2:
                        emit_l6_mms(s1, 6)
                    else:
                        emit_l6_mms(s1, 7)
                    emit_l6_add(s1, 2 * li)
                    emit_l6_add(s1, 2 * li + 1)
                if s3 is not None:
                    for c in range(3 * li + 4, 3 * li + 7):
                        emit_red(s3, c)

            if mlp:
                # L5: 64 -> 128 per group (zero-padded K=128 stationaries)
                blk3 = b3pool.tile([128, 2048], MD, tag="blk3",
                                   name=f"blk3_{it}")
                for gg in range(4):
                    half, sl = gg % 2, (gg // 2) * 512
                    p5 = mpsum.tile([128, 512], F32, tag="mp",
                                    name=f"p5_{gg}_{it}")
                    nc.tensor.matmul(p5[:], w5g_sb[:, half],
                                     cur[:, sl:sl + 512], start=True, stop=True)
                    nc.scalar.activation(blk3[:, gg * 512:(gg + 1) * 512],
                                         p5[:], Tanh, bias=bv_sb[:, 5:6])
                st["blk3"] = blk3
                st["t6"] = t6pool.tile([128, 4096], F32, tag="t6",
                                       name=f"t6_{it}")

            s3, s2, s1 = s2, s1, st

    _split_sync_waits(nc, max_waits)
    return nc


def _prep_in_maps(inputs):
    D = np.asarray(inputs["distance_matrices"], np.float32)
    Z = np.asarray(inputs["num_species_batch"], np.float32)
    ws = [np.asarray(inputs[f"W{i}"], np.float32) for i in range(7)]
    bs = [np.asarray(inputs[f"b{i}"], np.float32) for i in range(7)]

    x9, w = _host_features(D, Z)
    wd = _pack_weights(ws, bs)

    in_maps = []
    for c in range(NCORES):
        x9c = x9[c * BPC:(c + 1) * BPC].reshape(CENTERS, NPAIR, 9)
        wc = w[c * BPC:(c + 1) * BPC].reshape(CENTERS, NPAIR)
        x9d, wredd = _core_rows(x9c, wc)
        in_maps.append({"x9": x9d, "wred": wredd, **wd})
    return in_maps


def kernel(**inputs):
    from concourse.bass_utils import run_bass_kernel_spmd

    in_maps = _prep_in_maps(inputs)
    if "nc" not in _CACHE:
        _CACHE["nc"] = _build_nc()
    res = run_bass_kernel_spmd(_CACHE["nc"], in_maps, core_ids=list(range(NCORES)))
    outs = np.stack([res.results[c]["out"] for c in range(NCORES)])
    return np.ascontiguousarray(
        outs.reshape(NCORES, BPC, N, 256).reshape(B, N, 256).astype(np.float32))


# revision 18
# speedup vs baseline: 1.0681x; 1.0681x over previous
"""AEV triplet-MLP kernel for Trainium2 (8 NeuronCores, batch-parallel).

Host (numpy): builds per-(center, pair) 9-feature MLP inputs over all 496
j<k pairs (invalid pairs masked via zero weight) and the smooth-cutoff
weights. Device (Bass/Tile): runs the 7-layer tanh MLP on every row and the
weighted pair-sum, one core per 4 batch items.

Row layout per core (4 batch items = 128 centers, 496 pairs each):
  centers grouped 8-at-a-time (16 groups); within a group rows are
  interleaved (p, j) -> row = p*8 + j so every 128-row chunk holds rows of
  all 8 centers (16 p-values x 8 lanes). 63488 rows = 496 chunks of 128 =
  31 iterations of 2048 rows (2 "blocks" of 1024).

MLP layout: activations [features, rows]; 64-wide layers packed two
512-row groups per 128 partitions with block-diagonal weights. Final
128->256 layer runs "flipped" (stationary = activation chunk) so its
output is [rows, features]; the weighted sum over pairs is then a K=128
matmul per chunk with an [128, 8] weight slice (one column per center
lane), accumulated in PSUM over a group's 31 chunks.
"""

import ml_dtypes
import numpy as np

import concourse.bass as bass
import concourse.mybir as mybir
import concourse.tile as tile

CUTOFF = 3.5
EPS = 1e-7
B, N = 32, 32
NCORES = 8
BPC = B // NCORES            # batch items per core
CENTERS = BPC * N            # 128 centers per core
NPAIR = (N * (N - 1)) // 2   # 496 pairs (j < k), invalid ones zero-weighted
NGROUP = CENTERS // 8        # 16 groups of 8 centers
ROWS = CENTERS * NPAIR       # 63488 rows per core
NCHUNK = ROWS // 128         # 496
NITER = ROWS // 2048         # 31
NBLK = ROWS // 1024          # 62
CPG = NCHUNK // NGROUP       # 31 chunks per group
F32 = mybir.dt.float32
BF16 = mybir.dt.bfloat16
R32 = mybir.dt.float32r  # fp32 bytes, single-pass reduced-precision matmul

_PJ, _PK = np.triu_indices(N, k=1)  # pair index -> (j, k)

_CACHE = {}


def _host_features(D, Z):
    """D [B,N,N], Z [B,N] float32 -> x9 [B,N,NPAIR,9], w [B,N,NPAIR]."""
    jp, kp = _PJ, _PK
    R_ij = D[:, :, jp]                       # [B, N, P]
    R_ik = D[:, :, kp]
    R_jk = np.broadcast_to(D[:, jp, kp][:, None, :], R_ij.shape)
    z_i = np.broadcast_to(Z[:, :, None], R_ij.shape)
    z_j = np.broadcast_to(Z[:, jp][:, None, :], R_ij.shape)
    z_k = np.broadcast_to(Z[:, kp][:, None, :], R_ij.shape)

    def carnot(a, b, c):
        return (a * a + b * b - c * c) / np.maximum(2.0 * a * b, 1e-10)

    cos_i = carnot(R_ij, R_ik, R_jk)
    cos_j = carnot(R_ij, R_jk, R_ik)
    cos_k = carnot(R_ik, R_jk, R_ij)

    geo = np.stack(
        [R_ij + R_ik + R_jk,
         R_ij * R_ik + R_ij * R_jk + R_ik * R_jk,
         R_ij * R_ik * R_jk], axis=-1)
    chem = np.stack(
        [z_i + z_j + z_k,
         cos_i + cos_j + cos_k,
         z_i * (z_j + z_k) + z_j * z_k - cos_i * (cos_j + cos_k) - cos_j * cos_k,
         z_i * (cos_j + cos_k) + cos_i * (z_j + z_k) + z_j * cos_k + cos_j * z_k,
         z_i * (z_j * z_k - cos_j * cos_k) - cos_i * (z_j * cos_k + cos_j * z_k),
         z_i * (z_j * cos_k + cos_j * z_k) + cos_i * (z_j * z_k - cos_j * cos_k)],
        axis=-1)
    geo = geo / (np.linalg.norm(geo, axis=-1, keepdims=True).astype(np.float32) + EPS)
    chem = chem / (np.linalg.norm(chem, axis=-1, keepdims=True).astype(np.float32) + EPS)
    x9 = np.concatenate([geo, chem], axis=-1).astype(np.float32)

    def f_C(d):
        return 0.5 * np.cos(np.pi * d / CUTOFF) + 0.5

    within = (R_ij < CUTOFF) & (R_ik < CUTOFF)
    i_idx = np.arange(N)[None, :, None]
    valid = (jp[None, None, :] != i_idx) & (kp[None, None, :] != i_idx)
    w = (f_C(R_ij) * f_C(R_ik) * (within & valid)).astype(np.float32)
    return x9, w


def _core_rows(x9_core, w_core):
    """x9_core [CENTERS, P, 9], w_core [CENTERS, P] -> row-ordered arrays.

    Returns x9d [NBLK, 18, 512] (two 512-row groups stacked on partitions)
    and wredd [128, NCHUNK*8] (per-chunk [128, 8] reduction lhsT slices).
    """
    x9r = (x9_core.reshape(NGROUP, 8, NPAIR, 9)
           .transpose(0, 2, 1, 3).reshape(ROWS, 9))
    wr = (w_core.reshape(NGROUP, 8, NPAIR)
          .transpose(0, 2, 1).reshape(ROWS))

    t = x9r.reshape(NBLK, 2, 512, 9)
    x9d = np.ascontiguousarray(
        np.concatenate([t[:, 0], t[:, 1]], axis=2).transpose(0, 2, 1))

    wch = wr.reshape(NCHUNK, 128)
    M = np.zeros((NCHUNK, 128, 8), np.float32)
    q = np.arange(128)
    M[:, q, q % 8] = wch
    wredd = np.ascontiguousarray(M.transpose(1, 0, 2).reshape(128, NCHUNK * 8))
    return x9d.astype(np.float32), wredd


def _pack_weights(ws, bs):
    """ws/bs: lists of W0..W6, b0..b6 -> dict of device weight tensors."""
    W0, W1, W2, W3, W4, W5, W6 = ws
    b0, b1, b2, b3, b4, b5, b6 = bs

    w0d = np.zeros((18, 128), np.float32)
    w0d[0:9, 0:64] = W0
    w0d[9:18, 64:128] = W0

    wmid = np.zeros((128, 4, 128), np.float32)
    for li, Wl in enumerate([W1, W2, W3, W4]):
        wmid[0:64, li, 0:64] = Wl
        wmid[64:128, li, 64:128] = Wl

    w5g = np.zeros((128, 2, 128), np.float32)
    w5g[0:64, 0, :] = W5
    w5g[64:128, 1, :] = W5

    bvec = np.zeros((128, 6), np.float32)
    for li, bl in enumerate([b0, b1, b2, b3, b4]):
        bvec[0:64, li] = bl
        bvec[64:128, li] = bl
    bvec[:, 5] = b5

    b6b = np.ascontiguousarray(
        np.broadcast_to(np.concatenate([b6, b6]), (128, 512))).astype(np.float32)

    return {
        "w0d": w0d,
        "wmid": wmid,
        "w5g": w5g,
        "w6": np.ascontiguousarray(W6.astype(ml_dtypes.bfloat16)),
        "bvec": bvec,
        "b6b": b6b,
    }


def _split_sync_waits(nc, max_waits):
    """Hoist excess per-instruction semaphore waits onto preceding NoOps.

    The installed walrus build rejects instructions carrying more than
    `max_waits` sync waits ("Too many sync wait commands"); waiting on the
    same engine queue via dedicated NoOps first is semantically identical.
    """
    n = 0
    for fn in nc.m.functions:
        for bb in fn.blocks:
            il = bb.instructions
            out = []
            for inst in il:
                si = inst.sync_info
                if si is not None and si.on_wait and len(si.on_wait) > max_waits:
                    waits = list(si.on_wait)
                    for w in waits[:-max_waits]:
                        nop = mybir.InstNoOp(
                            name=nc.get_next_instruction_name(),
                            sync_info=mybir.SyncInfo(on_wait=[w], on_update=[]),
                            bass_nofuse=True,
                            engine=inst.engine,
                        )
                        nc.register_instruction(nop)
                        out.append(nop)
                        n += 1
                    si.on_wait = waits[-max_waits:]
                out.append(inst)
            il[:] = out
    return n


def _build_nc(max_waits=1, mm_dtype=R32):
    nc = bass.Bass()
    Tanh = mybir.ActivationFunctionType.Tanh
    MD = mm_dtype

    def r(ap):
        return ap

    x9 = nc.dram_tensor("x9", [NBLK, 18, 512], MD, kind="ExternalInput")
    wred = nc.dram_tensor("wred", [128, NCHUNK * 8], MD, kind="ExternalInput")
    w0d = nc.dram_tensor("w0d", [18, 128], MD, kind="ExternalInput")
    wmid = nc.dram_tensor("wmid", [128, 4, 128], MD, kind="ExternalInput")
    w5g = nc.dram_tensor("w5g", [128, 2, 128], MD, kind="ExternalInput")
    w6 = nc.dram_tensor("w6", [128, 256], BF16, kind="ExternalInput")
    bvec = nc.dram_tensor("bvec", [128, 6], F32, kind="ExternalInput")
    b6b = nc.dram_tensor("b6b", [128, 512], F32, kind="ExternalInput")
    out = nc.dram_tensor("out", [CENTERS, 256], F32, kind="ExternalOutput")

    with tile.TileContext(nc) as tc, \
         tc.tile_pool(name="const", bufs=1) as cpool, \
         tc.tile_pool(name="xin", bufs=4) as xpool, \
         tc.tile_pool(name="acts", bufs=3) as apool, \
         tc.tile_pool(name="blks", bufs=2) as bpool, \
         tc.tile_pool(name="blk3", bufs=2) as b3pool, \
         tc.tile_pool(name="t6", bufs=3) as t6pool, \
         tc.tile_pool(name="redsb", bufs=2) as rspool, \
         tc.tile_pool(name="mpsum", bufs=4, space="PSUM") as mpsum, \
         tc.tile_pool(name="lpsum", bufs=2, space="PSUM") as lpsum, \
         tc.tile_pool(name="rpsum", bufs=2, space="PSUM") as rpsum:

        w0_sb = cpool.tile([18, 128], MD)
        nc.sync.dma_start(w0_sb[:], w0d[:])
        wmid_sb = cpool.tile([128, 4, 128], MD)
        nc.sync.dma_start(wmid_sb[:], wmid[:])
        w5g_sb = cpool.tile([128, 2, 128], MD)
        nc.sync.dma_start(w5g_sb[:], w5g[:])
        w6_sb = cpool.tile([128, 256], BF16)
        nc.sync.dma_start(w6_sb[:], w6[:])
        bv_sb = cpool.tile([128, 6], F32)
        nc.sync.dma_start(bv_sb[:], bvec[:])
        b6b_sb = cpool.tile([128, 512], F32)
        nc.sync.dma_start(b6b_sb[:], b6b[:])
        wred_sb = cpool.tile([128, NCHUNK * 8], MD)
        nc.sync.dma_start(wred_sb[:], wred[:])

        red_tiles = {}

        def emit_l6_mms(st, cc):
            # two flipped-L6 chunk matmuls; the b6 add is deferred a section
            blk3 = st["blk3"]
            p6 = lpsum.tile([128, 512], F32, tag="lp",
                            name=f"p6_{st['it']}_{cc}")
            c0, c1 = 2 * cc, 2 * cc + 1
            nc.tensor.matmul(p6[:, 0:256], blk3[:, c0 * 128:(c0 + 1) * 128],
                             w6_sb[:], start=True, stop=True)
            nc.tensor.matmul(p6[:, 256:512], blk3[:, c1 * 128:(c1 + 1) * 128],
                             w6_sb[:], start=True, stop=True)
            st["p6"][cc] = p6

        def emit_l6_add(st, cc):
            nc.vector.tensor_add(out=st["t6"][:, cc * 512:(cc + 1) * 512],
                                 in0=st["p6"].pop(cc)[:], in1=b6b_sb[:])

        def emit_red(st, c):
            g = st["it"] * 16 + c
            G, loc = g // CPG, g % CPG
            if loc == 0:
                red_tiles[G] = rpsum.tile([8, 256], F32, tag="red",
                                          name=f"red_{G}")
            rp = red_tiles[G]
            nc.tensor.matmul(rp[:], wred_sb[:, g * 8:(g + 1) * 8],
                             st["t6h"][:, c * 256:(c + 1) * 256],
                             start=(loc == 0), stop=(loc == CPG - 1),
                             skip_group_check=True)
            if loc == CPG - 1:
                rs = rspool.tile([8, 256], F32, tag="rs", name=f"rs_{G}")
                nc.vector.tensor_copy(rs[:], rp[:])
                nc.sync.dma_start(out[G * 8:(G + 1) * 8, :], rs[:])
                del red_tiles[G]

        # 4-stage software pipeline. Iter `it` emits:
        #   s0=it:   MLP matmuls/tanhs (residual adds on GpSimd)
        #   s1=it-1: flipped-L6 chunk matmuls + b6 adds (PE/DVE filler)
        #   s2=it-2: the big L6 tanh (leads the ACT queue, input fully ready)
        #   s3=it-3: weighted-reduction matmuls (PE filler)
        # so the in-order PE/ACT queues always hold independent work between
        # layer-dependent ops (keeps PE dense -> HAM warm).
        s1 = s2 = s3 = None
        for it in range(NITER + 3):
            mlp = it < NITER
            st = None
            xres = blk1 = blk2 = cur = None
            if mlp:
                st = {"it": it}
                kA, kB = 2 * it, 2 * it + 1
                xtA = xpool.tile([18, 512], MD, tag="xt", name=f"xtA_{it}")
                nc.sync.dma_start(xtA[:], x9[kA])
                xtB = xpool.tile([18, 512], MD, tag="xt", name=f"xtB_{it}")
                nc.sync.dma_start(xtB[:], x9[kB])

            if s2 is not None:
                t6h = t6pool.tile([128, 4096], MD, tag="t6h",
                                  name=f"t6h_{s2['it']}")
                nc.scalar.activation(t6h[:, 0:2048], s2["t6"][:, 0:2048], Tanh)
                s2["t6h"] = t6h

            if mlp:
                # L0: 9 -> 64 (two row-groups block-diagonal packed)
                p0a = mpsum.tile([128, 512], F32, tag="mp", name=f"p0a_{it}")
                nc.tensor.matmul(p0a[:], w0_sb[:], xtA[:], start=True, stop=True)
                p0b = mpsum.tile([128, 512], F32, tag="mp", name=f"p0b_{it}")
                nc.tensor.matmul(p0b[:], w0_sb[:], xtB[:], start=True, stop=True)
                xres = apool.tile([128, 1024], MD, tag="xres",
                                  name=f"xres_{it}")
                nc.scalar.activation(xres[:, 0:512], p0a[:], Tanh,
                                     bias=bv_sb[:, 0:1])
                nc.scalar.activation(xres[:, 512:1024], p0b[:], Tanh,
                                     bias=bv_sb[:, 0:1])
                cur = xres

            if s1 is not None:
                s1["p6"] = {}
                emit_l6_mms(s1, 0)
                emit_l6_mms(s1, 1)
            if s3 is not None:
                for c in range(0, 4):
                    emit_red(s3, c)

            # L1..L4: 64 -> 64, interleaved with filler stages
            first_li = True
            for li in range(4):
                if mlp:
                    pla = mpsum.tile([128, 512], F32, tag="mp",
                                     name=f"p{li + 1}a_{it}")
                    nc.tensor.matmul(pla[:], wmid_sb[:, li],
                                     cur[:, 0:512], start=True, stop=True)
                    plb = mpsum.tile([128, 512], F32, tag="mp",
                                     name=f"p{li + 1}b_{it}")
                    nc.tensor.matmul(plb[:], wmid_sb[:, li],
                                     cur[:, 512:1024], start=True, stop=True)
                    h = apool.tile([128, 1024], MD, tag="h", name=f"h{li}_{it}")
                    nc.scalar.activation(h[:, 0:512], pla[:], Tanh,
                                         bias=bv_sb[:, li + 1:li + 2])
                    nc.scalar.activation(h[:, 512:1024], plb[:], Tanh,
                                         bias=bv_sb[:, li + 1:li + 2])
                    if li == 0:
                        blk1 = bpool.tile([128, 1024], MD, tag="blk1",
                                          name=f"blk1_{it}")
                        nc.vector.tensor_add(out=blk1[:], in0=h[:], in1=xres[:])
                        cur = blk1
                    elif li < 3:
                        cur = h
                    else:
                        blk2 = bpool.tile([128, 1024], MD, tag="blk2",
                                          name=f"blk2_{it}")
                        nc.vector.tensor_add(out=blk2[:], in0=h[:], in1=blk1[:])
                        cur = blk2
                if first_li and s2 is not None:
                    # second half of the big L6 tanh fills ACT's L1 wait
                    nc.scalar.activation(s2["t6h"][:, 2048:4096],
                                         s2["t6"][:, 2048:4096], Tanh)
                first_li = False
                if s1 is not None:
                    if li < 3:
                        emit_l6_mms(s1, 2 * li + 2)
                        emit_l6_mms(s1, 2 * li + 3)
                    emit_l6_add(s1, 2 * li)
                    emit_l6_add(s1, 2 * li + 1)
                if s3 is not None:
                    for c in range(3 * li + 4, 3 * li + 7):
                        emit_red(s3, c)

            if mlp:
                # L5: 64 -> 128 per group (zero-padded K=128 stationaries)
                blk3 = b3pool.tile([128, 2048], BF16, tag="blk3",
                                   name=f"blk3_{it}")
                for gg in range(4):
                    half, sl = gg % 2, (gg // 2) * 512
                    p5 = mpsum.tile([128, 512], F32, tag="mp",
                                    name=f"p5_{gg}_{it}")
                    nc.tensor.matmul(p5[:], w5g_sb[:, half],
                                     cur[:, sl:sl + 512], start=True, stop=True)
                    nc.scalar.activation(blk3[:, gg * 512:(gg + 1) * 512],
                                         p5[:], Tanh, bias=bv_sb[:, 5:6])
                st["blk3"] = blk3
                st["t6"] = t6pool.tile([128, 4096], F32, tag="t6",
                                       name=f"t6_{it}")

            s3, s2, s1 = s2, s1, st

    _split_sync_waits(nc, max_waits)
    return nc


def _prep_in_maps(inputs):
    D = np.asarray(inputs["distance_matrices"], np.float32)
    Z = np.asarray(inputs["num_species_batch"], np.float32)
    ws = [np.asarray(inputs[f"W{i}"], np.float32) for i in range(7)]
    bs = [np.asarray(inputs[f"b{i}"], np.float32) for i in range(7)]

    x9, w = _host_features(D, Z)
    wd = _pack_weights(ws, bs)

    in_maps = []
    for c in range(NCORES):
        x9c = x9[c * BPC:(c + 1) * BPC].reshape(CENTERS, NPAIR, 9)
        wc = w[c * BPC:(c + 1) * BPC].reshape(CENTERS, NPAIR)
        x9d, wredd = _core_rows(x9c, wc)
        in_maps.append({"x9": x9d, "wred": wredd, **wd})
    return in_maps


def kernel(**inputs):
    from concourse.bass_utils import run_bass_kernel_spmd

    in_maps = _prep_in_maps(inputs)
    if "nc" not in _CACHE:
        _CACHE["nc"] = _build_nc()
    res = run_bass_kernel_spmd(_CACHE["nc"], in_maps, core_ids=list(range(NCORES)))
    outs = np.stack([res.results[c]["out"] for c in range(NCORES)])
    return np.ascontiguousarray(
        outs.reshape(NCORES, BPC, N, 256).reshape(B, N, 256).astype(np.float32))
